# revision 5
# baseline (speedup 1.0000x reference)
"""Multi-head causal attention (B=4, T=2048, C=768, H=12, HS=64) on 8 trn2 cores.

Sharding: 48 (batch, head) units -> 6 per core. Core c: batch c//2, heads
6*(c%2) .. 6*(c%2)+6. Each core computes a partial output projection
y_partial[T, C] = sum over its 6 heads; host sums the two partials per batch
and adds the bias.

Per-core layout choices (all chosen so softmax reductions run along the free
dim and no big transposes are needed):
  xT      [C, T]        input, pre-transposed on host, bf16
  QT/KT   [64, T] f32   per head, stored pair-stacked in 128 partitions:
                        pairQ[p][0:64]=QT_{2p}, pairQ[p][64:128]=QT_{2p+1}
  scores  ST[tk, tq]    = KT_h^T-contraction matmul(lhsT=KT, rhs=QT), so the
                        P@V matmul can consume exp(ST) directly as rhs.
  softmax               no max-subtraction (scores are O(+-8); exp is safe in
                        f32), row sums via a ones-column appended to V.
  V       vaug[tk, h, 65] f32 (col 64 = 1.0)
  P@V     OTu[65, tq] = matmul(lhsT=vaug_tile, rhs=exp(ST)) accumulated over
                        tk tiles; row 64 = softmax denominator.
  norm    per tq tile: PE-transpose OTu -> [tq, 65], reciprocal + scale,
                        PE-transpose back -> OT[d, tq], pair-stacked (bf16)
  proj    y[tq, :] = sum_g matmul(lhsT=OT[:, g, tq], rhs=WpT[g])
"""

import numpy as np
import ml_dtypes

import concourse.bacc as bacc
import concourse.bass as bass
import concourse.tile as tile
from concourse import mybir
from concourse.masks import make_identity
from concourse import bass_utils

B, T, C = 4, 2048, 768
H, HS = 12, 64
HL = 6            # heads per core
NCT = C // 128    # 6 contraction tiles
NTT = T // 128    # 16 t tiles
NTC = T // 512    # 4 t chunks
NEG = -1.0e30
SCALE = 1.0 / 8.0  # 1/sqrt(HS)

F32 = mybir.dt.float32
BF16 = mybir.dt.bfloat16


def build_kernel(nc):
    xT = nc.dram_tensor("xT", [C, T], BF16, kind="ExternalInput").ap()
    wqk = nc.dram_tensor("wqk", [HL, NCT, 128, 128], BF16, kind="ExternalInput").ap()
    wv = nc.dram_tensor("wv", [NCT, 128, HL * HS], BF16, kind="ExternalInput").ap()
    wpt = nc.dram_tensor("wpt", [3, 128, C], BF16, kind="ExternalInput").ap()
    y = nc.dram_tensor("y", [T, C], F32, kind="ExternalOutput").ap()

    with tile.TileContext(nc) as tc:
        with (
            tc.tile_pool(name="consts", bufs=1) as consts,
            tc.tile_pool(name="xw", bufs=1) as xw,
            tc.tile_pool(name="qk_stage", bufs=3) as qk_stage,
            tc.tile_pool(name="pt", bufs=3) as ptp,
            tc.tile_pool(name="small", bufs=3) as small,
            tc.tile_pool(name="ysb", bufs=2) as ysbp,
            tc.tile_pool(name="ps_ab", bufs=1, space="PSUM") as ps_ab,
            tc.tile_pool(name="ps_st", bufs=2, space="PSUM") as ps_st,
            tc.tile_pool(name="ps_otu", bufs=1, space="PSUM") as ps_otu,
            tc.tile_pool(name="ps_t", bufs=2, space="PSUM") as ps_t,
        ):
            # ---------------- constants ----------------
            ident = consts.tile([128, 128], F32)
            make_identity(nc, ident)
            # causal mask for diagonal [tk, tq] subtiles: keep tq >= tk
            cmask = consts.tile([128, 128], F32)
            nc.gpsimd.memset(cmask, 0.0)
            nc.gpsimd.affine_select(
                out=cmask, in_=cmask,
                compare_op=mybir.AluOpType.is_ge,
                fill=NEG, base=0,
                pattern=[[1, 128]], channel_multiplier=-1,
            )  # keep where (-x + y) >= 0

            # ---------------- weights + x ----------------
            xt = []
            for ci in range(NCT):
                t_ = xw.tile([128, T], BF16, tag=f"xt{ci}", name=f"xt{ci}")
                nc.sync.dma_start(out=t_, in_=xT[ci * 128:(ci + 1) * 128, :])
                xt.append(t_)
            wqk_sb = []
            for h in range(HL):
                row = []
                for ci in range(NCT):
                    t_ = xw.tile([128, 128], BF16, tag=f"wqk{h}_{ci}", name=f"wqk{h}_{ci}")
                    nc.sync.dma_start(out=t_, in_=wqk[h, ci])
                    row.append(t_)
                wqk_sb.append(row)
            wv_sb = []
            for ci in range(NCT):
                t_ = xw.tile([128, HL * HS], BF16, tag=f"wv{ci}", name=f"wv{ci}")
                nc.sync.dma_start(out=t_, in_=wv[ci])
                wv_sb.append(t_)
            wpt_sb = []
            for g in range(3):
                t_ = consts.tile([128, C], BF16, tag=f"wpt{g}", name=f"wpt{g}")
                nc.sync.dma_start(out=t_, in_=wpt[g])
                wpt_sb.append(t_)

            # ---------------- phase 1a: V projection ----------------
            # vaug[:, tt, h, 0:64] = V tile, [..., 64] = 1.0
            vaug = consts.tile([128, NTT, HL, HS + 1], F32)
            nc.gpsimd.memset(vaug[:, :, :, HS:HS + 1], 1.0)
            for tt in range(NTT):
                ps = ps_ab.tile([128, HL * HS], F32, tag="b", name="psv")
                for ci in range(NCT):
                    nc.tensor.matmul(
                        ps, xt[ci][:, tt * 128:(tt + 1) * 128], wv_sb[ci],
                        start=(ci == 0), stop=(ci == NCT - 1),
                    )
                nc.vector.tensor_copy(
                    out=vaug[:, tt, :, 0:HS],
                    in_=ps.rearrange("p (h d) -> p h d", h=HL),
                )

            # ---------------- phase 1b: Q/K projections ----------------
            # pair p holds heads (2p, 2p+1); head parity e lives at
            # partitions 64e..64e+64 of pairQ/pairK.
            pairQ = [consts.tile([128, T], F32, tag=f"pq{p}", name=f"pq{p}") for p in range(3)]
            pairK = [consts.tile([128, T], F32, tag=f"pk{p}", name=f"pk{p}") for p in range(3)]
            for h in range(HL):
                p, e = divmod(h, 2)
                for m in range(NTC):
                    sl = slice(m * 512, (m + 1) * 512)
                    ps = ps_ab.tile([128, 512], F32, tag="a", name="psqk")
                    for ci in range(NCT):
                        nc.tensor.matmul(
                            ps, wqk_sb[h][ci], xt[ci][:, sl],
                            start=(ci == 0), stop=(ci == NCT - 1),
                        )
                    stg = qk_stage.tile([128, 512], F32, tag="qkstg")
                    nc.vector.tensor_copy(out=stg, in_=ps)
                    # Q half -> pairQ[p][64e:64e+64], K half -> pairK same rows
                    nc.sync.dma_start(
                        out=pairQ[p][64 * e:64 * e + 64, sl], in_=stg[0:64, :]
                    )
                    nc.sync.dma_start(
                        out=pairK[p][64 * e:64 * e + 64, sl], in_=stg[64:128, :]
                    )

            # ---------------- phase 2: attention ----------------
            # OT stored pair-stacked for the output projection: partition
            # rows of pair p: 0:64 head 2p dims, 64:128 head 2p+1 dims.
            otn = consts.tile([128, 3, T], BF16)
            for p in range(3):
                for m in range(NTC):
                    otu_ps = [
                        ps_otu.tile([HS + 1, 512], F32, tag=f"otu{e}",
                                    name=f"otu{e}")
                        for e in range(2)
                    ]
                    jmax = 4 * m + 3
                    for j in range(jmax + 1):
                        s0 = max(0, j - 4 * m)
                        w = 512 - 128 * s0
                        for e in range(2):
                            h = 2 * p + e
                            st = ps_st.tile([128, 512], F32, tag="st")
                            nc.tensor.matmul(
                                st[:, 128 * s0:512],
                                pairK[p][64 * e:64 * e + 64,
                                         j * 128:(j + 1) * 128],
                                pairQ[p][64 * e:64 * e + 64,
                                         m * 512 + 128 * s0:(m + 1) * 512],
                                start=True, stop=True,
                                tile_position=(64 * e, 0),
                            )
                            if j >= 4 * m:  # diagonal subtile needs the mask
                                nc.vector.tensor_add(
                                    out=st[:, 128 * s0:128 * s0 + 128],
                                    in0=st[:, 128 * s0:128 * s0 + 128],
                                    in1=cmask,
                                )
                            pt = ptp.tile([128, 512], F32, tag="pt")
                            nc.scalar.activation(
                                out=pt[:, 128 * s0:512],
                                in_=st[:, 128 * s0:512],
                                func=mybir.ActivationFunctionType.Exp,
                                scale=SCALE,
                            )
                            nc.tensor.matmul(
                                otu_ps[e][:, 128 * s0:512],
                                vaug[:, j, h, :],
                                pt[:, 128 * s0:512],
                                start=(j == 0), stop=(j == jmax),
                                skip_group_check=True,
                            )
                    # normalize + transpose per 128-wide tq tile
                    for s in range(4):
                        tq0 = m * 512 + s * 128
                        for e in range(2):
                            otu_sb = small.tile([HS + 1, 128], F32, tag="otusb")
                            nc.vector.tensor_copy(
                                out=otu_sb,
                                in_=otu_ps[e][:, s * 128:(s + 1) * 128],
                            )
                            t1 = ps_t.tile([128, HS + 1], F32, tag="tt", name="t1")
                            nc.tensor.transpose(
                                t1, otu_sb, ident[0:HS + 1, 0:HS + 1]
                            )
                            rinv = small.tile([128, 1], F32, tag="rinv")
                            nc.vector.reciprocal(out=rinv, in_=t1[:, HS:HS + 1])
                            onrm = small.tile([128, HS], F32, tag="onrm")
                            nc.vector.tensor_scalar_mul(
                                out=onrm, in0=t1[:, 0:HS], scalar1=rinv
                            )
                            t2 = ps_t.tile([HS, 128], F32, tag="tt", name="t2")
                            nc.tensor.transpose(t2, onrm, ident)
                            stg = small.tile([HS, 128], BF16, tag="t2stg")
                            nc.vector.tensor_copy(out=stg, in_=t2)
                            # cross-partition placement: head parity e lives
                            # at partitions 64e..64e+64 of otn
                            nc.sync.dma_start(
                                out=otn[64 * e:64 * e + HS, p, tq0:tq0 + 128],
                                in_=stg,
                            )

            # ---------------- phase 3: output projection ----------------
            for tt in range(NTT):
                y1 = ps_ab.tile([128, 512], F32, tag="a", name="y1")
                y2 = ps_ab.tile([128, 256], F32, tag="b", name="y2")
                for g in range(3):
                    lhs = otn[:, g, tt * 128:(tt + 1) * 128]
                    nc.tensor.matmul(
                        y1, lhs, wpt_sb[g][:, 0:512],
                        start=(g == 0), stop=(g == 2),
                    )
                    nc.tensor.matmul(
                        y2, lhs, wpt_sb[g][:, 512:768],
                        start=(g == 0), stop=(g == 2),
                    )
                ysb = ysbp.tile([128, C], F32, tag="ysb")
                nc.vector.tensor_copy(out=ysb[:, 0:512], in_=y1)
                nc.vector.tensor_copy(out=ysb[:, 512:768], in_=y2)
                nc.sync.dma_start(out=y[tt * 128:(tt + 1) * 128, :], in_=ysb)

    nc.compile()
    return nc


_NC_CACHE = {}


def get_nc():
    if "nc" not in _NC_CACHE:
        nc = bacc.Bacc(
            "TRN2", target_bir_lowering=False, debug=False, num_devices=8
        )
        _NC_CACHE["nc"] = build_kernel(nc)
    return _NC_CACHE["nc"]


def make_in_maps(x, Wq, Wk, Wv, Wp):
    x = np.asarray(x, dtype=np.float32)
    Wq = np.asarray(Wq, dtype=np.float32)
    Wk = np.asarray(Wk, dtype=np.float32)
    Wv = np.asarray(Wv, dtype=np.float32)
    Wp = np.asarray(Wp, dtype=np.float32)
    bf = ml_dtypes.bfloat16
    in_maps = []
    for c in range(8):
        b = c // 2
        hs = HL * (c % 2)
        xT = np.ascontiguousarray(x[b].T).astype(bf)
        wqk = np.empty((HL, NCT, 128, 128), dtype=bf)
        for h in range(HL):
            stacked = np.concatenate([Wq[hs + h], Wk[hs + h]], axis=1)  # [C,128]
            for ci in range(NCT):
                wqk[h, ci] = stacked[ci * 128:(ci + 1) * 128, :].astype(bf)
        # wv: [NCT, 128, HL*HS]
        wv_full = np.transpose(Wv[hs:hs + HL], (1, 0, 2)).reshape(C, HL * HS)
        wv = np.ascontiguousarray(
            wv_full.reshape(NCT, 128, HL * HS)
        ).astype(bf)
        # wpt: Wp[:, i_slice].T -> [384, C] -> [3, 128, C]
        wpt = np.ascontiguousarray(
            Wp[:, hs * HS:(hs + HL) * HS].T.reshape(3, 128, C)
        ).astype(bf)
        in_maps.append({"xT": xT, "wqk": wqk, "wv": wv, "wpt": wpt})
    return in_maps


def run(x, Wq, Wk, Wv, Wp, bp, trace=False):
    nc = get_nc()
    in_maps = make_in_maps(x, Wq, Wk, Wv, Wp)
    res = bass_utils.run_bass_kernel_spmd(
        nc, in_maps, core_ids=list(range(8)), trace=trace
    )
    y = np.zeros((B, T, C), dtype=np.float32)
    for c in range(8):
        y[c // 2] += res.results[c]["y"]
    y += np.asarray(bp, dtype=np.float32)
    return y, res


def kernel(x, Wq, Wk, Wv, Wp, bp):
    y, _ = run(x, Wq, Wk, Wv, Wp, bp)
    return y


def make_runner(nc):
    """Build the sharded PJRT callable once (mirrors the tail of
    bass2jax.run_bass_via_pjrt) so repeated timed executions don't re-trace.
    Returns (fn, prep) where prep(in_maps) device_puts the inputs and
    fn(device_inputs) -> per-core output dicts (blocking)."""
    import jax
    from jax.experimental.shard_map import shard_map
    from jax.sharding import Mesh, PartitionSpec, NamedSharding
    from concourse import mybir as _mybir
    from concourse.bass2jax import (
        _bass_exec_p, install_neuronx_cc_hook, partition_id_tensor,
    )

    install_neuronx_cc_hook()
    n_cores = 8
    partition_name = (
        nc.partition_id_tensor.name if nc.partition_id_tensor else None
    )
    in_names, out_names, out_avals = [], [], []
    for alloc in nc.m.functions[0].allocations:
        if not isinstance(alloc, _mybir.MemoryLocationSet):
            continue
        name = alloc.memorylocations[0].name
        if alloc.kind == "ExternalInput":
            if name != partition_name:
                in_names.append(name)
        elif alloc.kind == "ExternalOutput":
            out_names.append(name)
            out_avals.append(
                jax.core.ShapedArray(
                    tuple(alloc.tensor_shape), _mybir.dt.np(alloc.dtype)
                )
            )
    n_params = len(in_names)
    n_outs = len(out_avals)
    all_in_names = in_names + out_names
    if partition_name is not None:
        all_in_names.append(partition_name)

    def _body(*args):
        operands = list(args)
        if partition_name is not None:
            operands.append(partition_id_tensor())
        outs = _bass_exec_p.bind(
            *operands,
            out_avals=tuple(out_avals),
            in_names=tuple(all_in_names),
            out_names=tuple(out_names),
            lowering_input_output_aliases=(),
            sim_require_finite=True,
            sim_require_nnan=True,
            nc=nc,
        )
        return tuple(outs)

    devices = jax.devices()[:n_cores]
    mesh = Mesh(np.array(devices), ("core",))
    sharded = jax.jit(
        shard_map(
            _body, mesh=mesh,
            in_specs=(PartitionSpec("core"),) * (n_params + n_outs),
            out_specs=(PartitionSpec("core"),) * n_outs,
            check_rep=False,
        ),
        donate_argnums=tuple(range(n_params, n_params + n_outs)),
        keep_unused=True,
    )
    shd = NamedSharding(mesh, PartitionSpec("core"))

    def prep(in_maps):
        return [
            jax.device_put(
                np.concatenate([in_maps[c][nm] for c in range(n_cores)], axis=0),
                shd,
            )
            for nm in in_names
        ]

    def zeros():
        return [
            jax.device_put(
                np.zeros((n_cores * a.shape[0], *a.shape[1:]), a.dtype), shd
            )
            for a in out_avals
        ]

    def fn(dev_inputs, dev_zeros):
        outs = sharded(*dev_inputs, *dev_zeros)
        jax.block_until_ready(outs)
        return outs

    return fn, prep, zeros, out_names


# revision 8
# speedup vs baseline: 141.2155x; 141.2155x over previous
"""Multi-head causal attention (B=4, T=2048, C=768, H=12, HS=64) on 8 trn2 cores.

Sharding: 48 (batch, head) units -> 6 per core. Core c: batch c//2, heads
6*(c%2) .. 6*(c%2)+6. Each core computes a partial output projection
y_partial[T, C] = sum over its 6 heads; host sums the two partials per batch
and adds the bias.

Per-core layout choices (all chosen so softmax reductions run along the free
dim and no big transposes are needed):
  xT      [C, T]        input, pre-transposed on host, bf16
  QT/KT   [64, T] f32   per head, stored pair-stacked in 128 partitions:
                        pairQ[p][0:64]=QT_{2p}, pairQ[p][64:128]=QT_{2p+1}
  scores  ST[tk, tq]    = KT_h^T-contraction matmul(lhsT=KT, rhs=QT), so the
                        P@V matmul can consume exp(ST) directly as rhs.
  softmax               no max-subtraction (scores are O(+-8); exp is safe in
                        f32), row sums via a ones-column appended to V.
  V       vaug[tk, h, 65] f32 (col 64 = 1.0)
  P@V     OTu[65, tq] = matmul(lhsT=vaug_tile, rhs=exp(ST)) accumulated over
                        tk tiles; row 64 = softmax denominator.
  norm    per tq tile: PE-transpose OTu -> [tq, 65], reciprocal + scale,
                        PE-transpose back -> OT[d, tq], pair-stacked (bf16)
  proj    y[tq, :] = sum_g matmul(lhsT=OT[:, g, tq], rhs=WpT[g])
"""

import numpy as np
import ml_dtypes

import concourse.bacc as bacc
import concourse.bass as bass
import concourse.tile as tile
from concourse import mybir
from concourse.masks import make_identity
from concourse import bass_utils

B, T, C = 4, 2048, 768
H, HS = 12, 64
HL = 6            # heads per core
NCT = C // 128    # 6 contraction tiles
NTT = T // 128    # 16 t tiles
NTC = T // 512    # 4 t chunks
NEG = -1.0e30
SCALE = 1.0 / 8.0  # 1/sqrt(HS)

F32 = mybir.dt.float32
BF16 = mybir.dt.bfloat16


def build_kernel(nc, repeat=1):
    xT = nc.dram_tensor("xT", [C, T], BF16, kind="ExternalInput").ap()
    wqk = nc.dram_tensor("wqk", [HL, NCT, 128, 128], BF16, kind="ExternalInput").ap()
    wv = nc.dram_tensor("wv", [NCT, 128, HL * HS], BF16, kind="ExternalInput").ap()
    wpt = nc.dram_tensor("wpt", [3, 128, C], BF16, kind="ExternalInput").ap()
    y = nc.dram_tensor("y", [T, C], F32, kind="ExternalOutput").ap()

    with tile.TileContext(nc) as tc:
        with (
            tc.tile_pool(name="consts", bufs=1) as consts,
            tc.tile_pool(name="xw", bufs=1) as xw,
            tc.tile_pool(name="qk_stage", bufs=2) as qk_stage,
            tc.tile_pool(name="pt", bufs=3) as ptp,
            tc.tile_pool(name="small", bufs=3) as small,
            tc.tile_pool(name="ysb", bufs=2) as ysbp,
            # PSUM: st2 (2 slots x 2 banks) + otu2 (1 slot x 2 banks) +
            # tt (2 slots x 1 bank, shared by psv/psqk/t1/t2/y1/y2) = 8 banks
            tc.tile_pool(name="ps_st", bufs=2, space="PSUM") as ps_st,
            tc.tile_pool(name="ps_otu", bufs=1, space="PSUM") as ps_otu,
            tc.tile_pool(name="ps_t", bufs=2, space="PSUM") as ps_t,
        ):
            # ---------------- constants ----------------
            ident = consts.tile([128, 128], F32)
            make_identity(nc, ident)
            # causal mask for diagonal [tk, tq] subtiles: keep tq >= tk
            cmask = consts.tile([128, 128], F32)
            nc.gpsimd.memset(cmask, 0.0)
            nc.gpsimd.affine_select(
                out=cmask, in_=cmask,
                compare_op=mybir.AluOpType.is_ge,
                fill=NEG, base=0,
                pattern=[[1, 128]], channel_multiplier=-1,
            )  # keep where (-x + y) >= 0

            # ---------------- weights + x ----------------
            xt = []
            for ci in range(NCT):
                t_ = xw.tile([128, T], BF16, tag=f"xt{ci}", name=f"xt{ci}")
                nc.sync.dma_start(out=t_, in_=xT[ci * 128:(ci + 1) * 128, :])
                xt.append(t_)
            wqk_sb = []
            for h in range(HL):
                row = []
                for ci in range(NCT):
                    t_ = xw.tile([128, 128], BF16, tag=f"wqk{h}_{ci}",
                                 name=f"wqk{h}_{ci}")
                    nc.sync.dma_start(out=t_, in_=wqk[h, ci])
                    row.append(t_)
                wqk_sb.append(row)
            wv_sb = []
            for ci in range(NCT):
                t_ = xw.tile([128, HL * HS], BF16, tag=f"wv{ci}", name=f"wv{ci}")
                nc.sync.dma_start(out=t_, in_=wv[ci])
                wv_sb.append(t_)
            wpt_sb = []
            for g in range(3):
                t_ = consts.tile([128, C], BF16, tag=f"wpt{g}", name=f"wpt{g}")
                nc.sync.dma_start(out=t_, in_=wpt[g])
                wpt_sb.append(t_)

            import contextlib
            rep_ctx = (
                tc.For_i(0, repeat, 1,
                         hint_engines=(mybir.EngineType.PE,
                                       mybir.EngineType.DVE,
                                       mybir.EngineType.Activation,
                                       mybir.EngineType.SP,
                                       mybir.EngineType.Pool))
                if repeat > 1 else contextlib.nullcontext()
            )
            with rep_ctx:
                build_phases(nc, tc, consts, xw, qk_stage, ptp, small, ysbp,
                             ps_st, ps_otu, ps_t,
                             ident, cmask, xt, wqk_sb, wv_sb, wpt_sb,
                             xT, wqk, wv, wpt, y)

    nc.compile()
    return nc


def build_phases(nc, tc, consts, xw, qk_stage, ptp, small, ysbp,
                 ps_st, ps_otu, ps_t,
                 ident, cmask, xt, wqk_sb, wv_sb, wpt_sb,
                 xT, wqk, wv, wpt, y):
    if True:
        if True:
            # ---------------- phase 1a: V projection ----------------
            # vaug[:, tt, h, 0:64] = V tile, [..., 64] = 1.0
            vaug = consts.tile([128, NTT, HL, HS + 1], F32)
            nc.gpsimd.memset(vaug[:, :, :, HS:HS + 1], 1.0)
            for tt in range(NTT):
                ps = ps_t.tile([128, HL * HS], F32, tag="tt", name="psv")
                for ci in range(NCT):
                    nc.tensor.matmul(
                        ps, xt[ci][:, tt * 128:(tt + 1) * 128], wv_sb[ci],
                        start=(ci == 0), stop=(ci == NCT - 1),
                    )
                nc.vector.tensor_copy(
                    out=vaug[:, tt, :, 0:HS],
                    in_=ps.rearrange("p (h d) -> p h d", h=HL),
                )

            # ---------------- phase 1b: Q/K projections ----------------
            # pair p holds heads (2p, 2p+1); head parity e lives at
            # partitions 64e..64e+64 of pairQ/pairK.
            pairQ = [consts.tile([128, T], F32, tag=f"pq{p}", name=f"pq{p}")
                     for p in range(3)]
            pairK = [consts.tile([128, T], F32, tag=f"pk{p}", name=f"pk{p}")
                     for p in range(3)]
            for h in range(HL):
                p, e = divmod(h, 2)
                stg = qk_stage.tile([128, T], F32, tag="qkstg", name="qkstg")
                for m in range(NTC):
                    sl = slice(m * 512, (m + 1) * 512)
                    ps = ps_t.tile([128, 512], F32, tag="tt", name="psqk")
                    for ci in range(NCT):
                        nc.tensor.matmul(
                            ps, wqk_sb[h][ci], xt[ci][:, sl],
                            start=(ci == 0), stop=(ci == NCT - 1),
                        )
                    nc.vector.tensor_copy(out=stg[:, sl], in_=ps)
                # Q half -> pairQ[p][64e:64e+64], K half -> pairK same rows
                nc.gpsimd.dma_start(
                    out=pairQ[p][64 * e:64 * e + 64, :], in_=stg[0:64, :]
                )
                nc.gpsimd.dma_start(
                    out=pairK[p][64 * e:64 * e + 64, :], in_=stg[64:128, :]
                )

            # ---------------- phase 2: attention ----------------
            # OT stored pair-stacked for the output projection: partition
            # rows of pair p: 0:64 head 2p dims, 64:128 head 2p+1 dims.
            otn = consts.tile([128, 3, T], BF16)
            for p in range(3):
                for m in range(NTC):
                    otu_ps = ps_otu.tile([HS + 1, 2, 512], F32, tag="otu",
                                         name="otu")
                    jmax = 4 * m + 3
                    for j in range(jmax + 1):
                        s0 = max(0, j - 4 * m)
                        st = ps_st.tile([128, 2, 512], F32, tag="st", name="st")
                        for e in range(2):
                            nc.tensor.matmul(
                                st[:, e, 128 * s0:512],
                                pairK[p][64 * e:64 * e + 64,
                                         j * 128:(j + 1) * 128],
                                pairQ[p][64 * e:64 * e + 64,
                                         m * 512 + 128 * s0:(m + 1) * 512],
                                start=True, stop=True,
                                tile_position=(64 * e, 0),
                            )
                            if j >= 4 * m:  # diagonal subtile needs the mask
                                nc.vector.tensor_add(
                                    out=st[:, e, 128 * s0:128 * s0 + 128],
                                    in0=st[:, e, 128 * s0:128 * s0 + 128],
                                    in1=cmask,
                                )
                        pt = ptp.tile([128, 2, 512], F32, tag="pt", name="pt")
                        # one fused exp over both heads (2-bank strided AP)
                        nc.scalar.activation(
                            out=pt[:, :, 128 * s0:512],
                            in_=st[:, :, 128 * s0:512],
                            func=mybir.ActivationFunctionType.Exp,
                            scale=SCALE,
                        )
                        for e in range(2):
                            nc.tensor.matmul(
                                otu_ps[:, e, 128 * s0:512],
                                vaug[:, j, 2 * p + e, :],
                                pt[:, e, 128 * s0:512],
                                start=(j == 0), stop=(j == jmax),
                                skip_group_check=True,
                            )
                    # normalize + transpose per 128-wide tq tile
                    otu_sb = small.tile([HS + 1, 2, 512], F32, tag="otusb",
                                        name="otusb")
                    nc.vector.tensor_copy(out=otu_sb, in_=otu_ps)
                    for e in range(2):
                        stg2 = small.tile([HS, 512], BF16, tag="t2stg",
                                          name="t2stg")
                        for s in range(4):
                            t1 = ps_t.tile([128, HS + 1], F32, tag="tt",
                                           name="t1")
                            nc.tensor.transpose(
                                t1, otu_sb[:, e, s * 128:(s + 1) * 128],
                                ident[0:HS + 1, 0:HS + 1],
                            )
                            rinv = small.tile([128, 1], F32, tag="rinv",
                                              name="rinv")
                            nc.vector.reciprocal(out=rinv, in_=t1[:, HS:HS + 1])
                            onrm = small.tile([128, HS], F32, tag="onrm",
                                              name="onrm")
                            nc.vector.tensor_scalar_mul(
                                out=onrm, in0=t1[:, 0:HS], scalar1=rinv
                            )
                            t2 = ps_t.tile([HS, 128], F32, tag="tt", name="t2")
                            nc.tensor.transpose(t2, onrm, ident)
                            nc.vector.tensor_copy(
                                out=stg2[:, s * 128:(s + 1) * 128], in_=t2
                            )
                        # cross-partition placement: head parity e lives at
                        # partitions 64e..64e+64 of otn; one DMA per (p,m,e)
                        nc.gpsimd.dma_start(
                            out=otn[64 * e:64 * e + HS, p,
                                    m * 512:(m + 1) * 512],
                            in_=stg2,
                        )

            # ---------------- phase 3: output projection ----------------
            for tt in range(NTT):
                y1 = ps_t.tile([128, 512], F32, tag="tt", name="y1")
                y2 = ps_t.tile([128, 256], F32, tag="tt", name="y2")
                for g in range(3):
                    lhs = otn[:, g, tt * 128:(tt + 1) * 128]
                    nc.tensor.matmul(
                        y1, lhs, wpt_sb[g][:, 0:512],
                        start=(g == 0), stop=(g == 2),
                    )
                    nc.tensor.matmul(
                        y2, lhs, wpt_sb[g][:, 512:768],
                        start=(g == 0), stop=(g == 2),
                    )
                ysb = ysbp.tile([128, C], F32, tag="ysb", name="ysb")
                nc.vector.tensor_copy(out=ysb[:, 0:512], in_=y1)
                nc.vector.tensor_copy(out=ysb[:, 512:768], in_=y2)
                nc.sync.dma_start(out=y[tt * 128:(tt + 1) * 128, :], in_=ysb)


_NC_CACHE = {}


def get_nc(repeat=1):
    if repeat not in _NC_CACHE:
        nc = bacc.Bacc(
            "TRN2", target_bir_lowering=False, debug=False, num_devices=8
        )
        _NC_CACHE[repeat] = build_kernel(nc, repeat=repeat)
    return _NC_CACHE[repeat]


def make_in_maps(x, Wq, Wk, Wv, Wp):
    x = np.asarray(x, dtype=np.float32)
    Wq = np.asarray(Wq, dtype=np.float32)
    Wk = np.asarray(Wk, dtype=np.float32)
    Wv = np.asarray(Wv, dtype=np.float32)
    Wp = np.asarray(Wp, dtype=np.float32)
    bf = ml_dtypes.bfloat16
    in_maps = []
    for c in range(8):
        b = c // 2
        hs = HL * (c % 2)
        xT = np.ascontiguousarray(x[b].T).astype(bf)
        wqk = np.empty((HL, NCT, 128, 128), dtype=bf)
        for h in range(HL):
            stacked = np.concatenate([Wq[hs + h], Wk[hs + h]], axis=1)  # [C,128]
            for ci in range(NCT):
                wqk[h, ci] = stacked[ci * 128:(ci + 1) * 128, :].astype(bf)
        # wv: [NCT, 128, HL*HS]
        wv_full = np.transpose(Wv[hs:hs + HL], (1, 0, 2)).reshape(C, HL * HS)
        wv = np.ascontiguousarray(
            wv_full.reshape(NCT, 128, HL * HS)
        ).astype(bf)
        # wpt: Wp[:, i_slice].T -> [384, C] -> [3, 128, C]
        wpt = np.ascontiguousarray(
            Wp[:, hs * HS:(hs + HL) * HS].T.reshape(3, 128, C)
        ).astype(bf)
        in_maps.append({"xT": xT, "wqk": wqk, "wv": wv, "wpt": wpt})
    return in_maps


def run(x, Wq, Wk, Wv, Wp, bp, trace=False):
    nc = get_nc()
    in_maps = make_in_maps(x, Wq, Wk, Wv, Wp)
    res = bass_utils.run_bass_kernel_spmd(
        nc, in_maps, core_ids=list(range(8)), trace=trace
    )
    y = np.zeros((B, T, C), dtype=np.float32)
    for c in range(8):
        y[c // 2] += res.results[c]["y"]
    y += np.asarray(bp, dtype=np.float32)
    return y, res


def kernel(x, Wq, Wk, Wv, Wp, bp):
    y, _ = run(x, Wq, Wk, Wv, Wp, bp)
    return y


def make_runner(nc):
    """Build the sharded PJRT callable once (mirrors the tail of
    bass2jax.run_bass_via_pjrt) so repeated timed executions don't re-trace.
    Returns (fn, prep) where prep(in_maps) device_puts the inputs and
    fn(device_inputs) -> per-core output dicts (blocking)."""
    import jax
    from jax.experimental.shard_map import shard_map
    from jax.sharding import Mesh, PartitionSpec, NamedSharding
    from concourse import mybir as _mybir
    from concourse.bass2jax import (
        _bass_exec_p, install_neuronx_cc_hook, partition_id_tensor,
    )

    install_neuronx_cc_hook()
    n_cores = 8
    partition_name = (
        nc.partition_id_tensor.name if nc.partition_id_tensor else None
    )
    in_names, out_names, out_avals = [], [], []
    for alloc in nc.m.functions[0].allocations:
        if not isinstance(alloc, _mybir.MemoryLocationSet):
            continue
        name = alloc.memorylocations[0].name
        if alloc.kind == "ExternalInput":
            if name != partition_name:
                in_names.append(name)
        elif alloc.kind == "ExternalOutput":
            out_names.append(name)
            out_avals.append(
                jax.core.ShapedArray(
                    tuple(alloc.tensor_shape), _mybir.dt.np(alloc.dtype)
                )
            )
    n_params = len(in_names)
    n_outs = len(out_avals)
    all_in_names = in_names + out_names
    if partition_name is not None:
        all_in_names.append(partition_name)

    def _body(*args):
        operands = list(args)
        if partition_name is not None:
            operands.append(partition_id_tensor())
        outs = _bass_exec_p.bind(
            *operands,
            out_avals=tuple(out_avals),
            in_names=tuple(all_in_names),
            out_names=tuple(out_names),
            lowering_input_output_aliases=(),
            sim_require_finite=True,
            sim_require_nnan=True,
            nc=nc,
        )
        return tuple(outs)

    devices = jax.devices()[:n_cores]
    mesh = Mesh(np.array(devices), ("core",))
    sharded = jax.jit(
        shard_map(
            _body, mesh=mesh,
            in_specs=(PartitionSpec("core"),) * (n_params + n_outs),
            out_specs=(PartitionSpec("core"),) * n_outs,
            check_rep=False,
        ),
        donate_argnums=tuple(range(n_params, n_params + n_outs)),
        keep_unused=True,
    )
    shd = NamedSharding(mesh, PartitionSpec("core"))

    def prep(in_maps):
        return [
            jax.device_put(
                np.concatenate([in_maps[c][nm] for c in range(n_cores)], axis=0),
                shd,
            )
            for nm in in_names
        ]

    def zeros():
        return [
            jax.device_put(
                np.zeros((n_cores * a.shape[0], *a.shape[1:]), a.dtype), shd
            )
            for a in out_avals
        ]

    def fn(dev_inputs, dev_zeros):
        outs = sharded(*dev_inputs, *dev_zeros)
        jax.block_until_ready(outs)
        return outs

    def make_loop_fn(n_iters):
        def _body_n(*args):
            ins = args[:n_params]
            carry = tuple(args[n_params:])

            def step(i, carry):
                operands = list(ins) + list(carry)
                if partition_name is not None:
                    operands.append(partition_id_tensor())
                outs = _bass_exec_p.bind(
                    *operands,
                    out_avals=tuple(out_avals),
                    in_names=tuple(all_in_names),
                    out_names=tuple(out_names),
                    lowering_input_output_aliases=(),
                    sim_require_finite=True,
                    sim_require_nnan=True,
                    nc=nc,
                )
                return tuple(outs)

            return jax.lax.fori_loop(0, n_iters, step, carry)

        looped = jax.jit(
            shard_map(
                _body_n, mesh=mesh,
                in_specs=(PartitionSpec("core"),) * (n_params + n_outs),
                out_specs=(PartitionSpec("core"),) * n_outs,
                check_rep=False,
            ),
            donate_argnums=tuple(range(n_params, n_params + n_outs)),
            keep_unused=True,
        )

        def run_n(dev_inputs, dev_zeros):
            outs = looped(*dev_inputs, *dev_zeros)
            jax.block_until_ready(outs)
            return outs

        return run_n

    return fn, prep, zeros, out_names, make_loop_fn


# revision 14
# speedup vs baseline: 371.4955x; 2.6307x over previous
"""Multi-head causal attention (B=4, T=2048, C=768, H=12, HS=64) on 8 trn2 cores.

Sharding: 48 (batch, head) units -> 6 per core. Core c: batch c//2, heads
6*(c%2) .. 6*(c%2)+6. Each core computes a partial output projection
y_partial[T, C] = sum over its 6 heads; host sums the two partials per batch
and adds the bias.

Per-core layout choices (all chosen so softmax reductions run along the free
dim and no big transposes are needed):
  xT      [C, T]        input, pre-transposed on host, bf16
  QT/KT   [64, T] f32   per head, stored pair-stacked in 128 partitions:
                        pairQ[p][0:64]=QT_{2p}, pairQ[p][64:128]=QT_{2p+1}
  scores  ST[tk, tq]    = KT_h^T-contraction matmul(lhsT=KT, rhs=QT), so the
                        P@V matmul can consume exp(ST) directly as rhs.
  softmax               no max-subtraction (scores are O(+-8); exp is safe in
                        f32), row sums via a ones-column appended to V.
  V       vaug[tk, h, 65] f32 (col 64 = 1.0)
  P@V     OTu[65, tq] = matmul(lhsT=vaug_tile, rhs=exp(ST)) accumulated over
                        tk tiles; row 64 = softmax denominator.
  norm    per tq tile: PE-transpose OTu -> [tq, 65], reciprocal + scale,
                        PE-transpose back -> OT[d, tq], pair-stacked (bf16)
  proj    y[tq, :] = sum_g matmul(lhsT=OT[:, g, tq], rhs=WpT[g])
"""

import numpy as np
import ml_dtypes

import concourse.bacc as bacc
import concourse.bass as bass
import concourse.tile as tile
from concourse import mybir
from concourse.masks import make_identity
from concourse import bass_utils

B, T, C = 4, 2048, 768
H, HS = 12, 64
HL = 6            # heads per core
NCT = C // 128    # 6 contraction tiles
NTT = T // 128    # 16 t tiles
NTC = T // 512    # 4 t chunks
NEG = -1.0e30
SCALE = 1.0 / 8.0  # 1/sqrt(HS)

F32 = mybir.dt.float32
BF16 = mybir.dt.bfloat16


def build_kernel(nc, repeat=1, phases=("v", "qk", "attn", "norm", "proj")):
    xT = nc.dram_tensor("xT", [C, T], BF16, kind="ExternalInput").ap()
    wqk = nc.dram_tensor("wqk", [HL, NCT, 128, 128], BF16, kind="ExternalInput").ap()
    wv = nc.dram_tensor("wv", [NCT, 128, HL * HS], BF16, kind="ExternalInput").ap()
    wpt = nc.dram_tensor("wpt", [3, 128, C], BF16, kind="ExternalInput").ap()
    y = nc.dram_tensor("y", [T, C], F32, kind="ExternalOutput").ap()

    with tile.TileContext(nc) as tc:
        with (
            tc.tile_pool(name="consts", bufs=1) as consts,
            tc.tile_pool(name="xw", bufs=1) as xw,
            tc.tile_pool(name="qk_stage", bufs=2) as qk_stage,
            tc.tile_pool(name="pt", bufs=16) as ptp,
            tc.tile_pool(name="small", bufs=3) as small,
            tc.tile_pool(name="ysb", bufs=2) as ysbp,
            # PSUM: st2 (2 slots x 2 banks) + otu2 (1 slot x 2 banks) +
            # tt (2 slots x 1 bank, shared by psv/psqk/t1/t2/y1/y2) = 8 banks
            tc.tile_pool(name="ps_st", bufs=2, space="PSUM") as ps_st,
            tc.tile_pool(name="ps_otu", bufs=1, space="PSUM") as ps_otu,
            tc.tile_pool(name="ps_t", bufs=2, space="PSUM") as ps_t,
        ):
            # ---------------- constants ----------------
            ident = consts.tile([128, 128], F32)
            make_identity(nc, ident)
            # causal mask for diagonal [tk, tq] subtiles: keep tq >= tk
            cmask = consts.tile([128, 128], F32)
            nc.gpsimd.memset(cmask, 0.0)
            nc.gpsimd.affine_select(
                out=cmask, in_=cmask,
                compare_op=mybir.AluOpType.is_ge,
                fill=NEG, base=0,
                pattern=[[1, 128]], channel_multiplier=-1,
            )  # keep where (-x + y) >= 0

            # ---------------- weights + x ----------------
            xt = []
            for ci in range(NCT):
                t_ = xw.tile([128, T], BF16, tag=f"xt{ci}", name=f"xt{ci}")
                nc.sync.dma_start(out=t_, in_=xT[ci * 128:(ci + 1) * 128, :])
                xt.append(t_)
            wqk_sb = []
            for h in range(HL):
                row = []
                for ci in range(NCT):
                    t_ = xw.tile([128, 128], BF16, tag=f"wqk{h}_{ci}",
                                 name=f"wqk{h}_{ci}")
                    nc.sync.dma_start(out=t_, in_=wqk[h, ci])
                    row.append(t_)
                wqk_sb.append(row)
            wv_sb = []
            for ci in range(NCT):
                t_ = xw.tile([128, HL * HS], BF16, tag=f"wv{ci}", name=f"wv{ci}")
                nc.sync.dma_start(out=t_, in_=wv[ci])
                wv_sb.append(t_)
            wpt_sb = []
            for g in range(3):
                t_ = consts.tile([128, C], BF16, tag=f"wpt{g}", name=f"wpt{g}")
                nc.sync.dma_start(out=t_, in_=wpt[g])
                wpt_sb.append(t_)

            # persistent tensors (allocated once; loop iterations rewrite)
            vaug = consts.tile([128, NTT, HL, HS + 1], BF16)
            nc.gpsimd.memset(vaug[:, :, :, HS:HS + 1], 1.0)
            pairQ = [consts.tile([128, T], BF16, tag=f"pq{p}", name=f"pq{p}")
                     for p in range(3)]
            pairK = [consts.tile([128, T], BF16, tag=f"pk{p}", name=f"pk{p}")
                     for p in range(3)]
            otn = consts.tile([128, 3, T], BF16)
            if "v" not in phases:
                nc.gpsimd.memset(vaug[:, :, :, 0:HS], 0.0)
            if "qk" not in phases:
                for p in range(3):
                    nc.gpsimd.memset(pairQ[p], 0.0)
                    nc.gpsimd.memset(pairK[p], 0.0)
            if "norm" not in phases:
                nc.gpsimd.memset(otn, 0.0)

            import contextlib
            rep_ctx = (
                tc.For_i(0, repeat, 1,
                         hint_engines=(mybir.EngineType.PE,
                                       mybir.EngineType.DVE,
                                       mybir.EngineType.Activation,
                                       mybir.EngineType.SP,
                                       mybir.EngineType.Pool))
                if repeat > 1 else contextlib.nullcontext()
            )
            with rep_ctx:
                build_phases(nc, tc, consts, xw, qk_stage, ptp, small, ysbp,
                             ps_st, ps_otu, ps_t,
                             ident, cmask, xt, wqk_sb, wv_sb, wpt_sb,
                             vaug, pairQ, pairK, otn, y, phases)

    nc.compile()
    return nc


def build_phases(nc, tc, consts, xw, qk_stage, ptp, small, ysbp,
                 ps_st, ps_otu, ps_t,
                 ident, cmask, xt, wqk_sb, wv_sb, wpt_sb,
                 vaug, pairQ, pairK, otn, y,
                 phases=("v", "qk", "attn", "norm", "proj")):
    if True:
        if True:
            ones_rows = consts.tile([128, HS + 1], F32)
            nc.gpsimd.memset(ones_rows, 1.0)

            # ---------------- phase 1a: V projection ----------------
            # vaug[:, tt, h, 0:64] = V tile, [..., 64] = 1.0
            for tt in range(NTT if "v" in phases else 0):
                ps = ps_t.tile([128, HL * HS], F32, tag="tt", name="psv")
                for ci in range(NCT):
                    nc.tensor.matmul(
                        ps, xt[ci][:, tt * 128:(tt + 1) * 128], wv_sb[ci],
                        start=(ci == 0), stop=(ci == NCT - 1),
                    )
                nc.vector.tensor_copy(
                    out=vaug[:, tt, :, 0:HS],
                    in_=ps.rearrange("p (h d) -> p h d", h=HL),
                )

            # ---------------- phase 1b: Q/K projections ----------------
            # pair p holds heads (2p, 2p+1); head parity e lives at
            # partitions 64e..64e+64 of pairQ/pairK.
            for h in range(HL if "qk" in phases else 0):
                p, e = divmod(h, 2)
                stg = qk_stage.tile([128, T], BF16, tag="qkstg", name="qkstg")
                for m in range(NTC):
                    sl = slice(m * 512, (m + 1) * 512)
                    ps = ps_t.tile([128, 512], F32, tag="tt", name="psqk")
                    for ci in range(NCT):
                        nc.tensor.matmul(
                            ps, wqk_sb[h][ci], xt[ci][:, sl],
                            start=(ci == 0), stop=(ci == NCT - 1),
                        )
                    nc.vector.tensor_copy(out=stg[:, sl], in_=ps)
                # Q half -> pairQ[p][64e:64e+64], K half -> pairK same rows
                nc.sync.dma_start(
                    out=pairQ[p][64 * e:64 * e + 64, :], in_=stg[0:64, :]
                )
                nc.sync.dma_start(
                    out=pairK[p][64 * e:64 * e + 64, :], in_=stg[64:128, :]
                )

            # ---------------- phase 2: attention ----------------
            # OT stored pair-stacked for the output projection: partition
            # rows of pair p: 0:64 head 2p dims, 64:128 head 2p+1 dims.
            for p in range(3 if ({"attn", "norm", "sc", "exp", "otu"} & set(phases)) else 0):
                for m in range(NTC):
                    otu_ps = ps_otu.tile([HS + 1, 2, 512], F32, tag="otu",
                                         name="otu")
                    if "norm" in phases and not (
                        "attn" in phases or "otu" in phases
                    ):
                        nc.vector.memset(otu_ps, 1.0)
                    jmax = 4 * m + 3
                    do_sc = "attn" in phases or "sc" in phases
                    do_exp = "attn" in phases or "exp" in phases
                    do_otu = "attn" in phases or "otu" in phases
                    pts = []
                    for j in range((jmax + 1) if do_sc else 0):
                        s0 = max(0, j - 4 * m)
                        st = ps_st.tile([128, 2, 512], F32, tag="st", name="st")
                        for e in range(2):
                            nc.tensor.matmul(
                                st[:, e, 128 * s0:512],
                                pairK[p][64 * e:64 * e + 64,
                                         j * 128:(j + 1) * 128],
                                pairQ[p][64 * e:64 * e + 64,
                                         m * 512 + 128 * s0:(m + 1) * 512],
                                start=True, stop=True,
                                tile_position=(64 * e, 0),
                            )
                        if not do_exp:
                            continue
                        pt = ptp.tile([128, 2, 512], BF16, tag="pt", name="pt")
                        pts.append(pt)
                        # one fused exp over both heads (2-bank strided AP)
                        nc.scalar.activation(
                            out=pt[:, :, 128 * s0:512],
                            in_=st[:, :, 128 * s0:512],
                            func=mybir.ActivationFunctionType.Exp,
                            scale=SCALE,
                        )
                        if j >= 4 * m:
                            # zero the below-diagonal triangle of the diagonal
                            # subtile for both heads (keep where tq >= tk)
                            nc.gpsimd.affine_select(
                                out=pt[:, :, 128 * s0:128 * s0 + 128],
                                in_=pt[:, :, 128 * s0:128 * s0 + 128],
                                compare_op=mybir.AluOpType.is_ge,
                                fill=0.0, base=0,
                                pattern=[[0, 2], [1, 128]],
                                channel_multiplier=-1,
                            )
                    # separate P@V loop: PE never stalls on ACT here
                    for j in range((jmax + 1) if do_otu else 0):
                        s0 = max(0, j - 4 * m)
                        for e in range(2):
                            nc.tensor.matmul(
                                otu_ps[:, e, 128 * s0:512],
                                vaug[:, j, 2 * p + e, :],
                                pts[j][:, e, 128 * s0:512],
                                start=(j == 0), stop=(j == jmax),
                                skip_group_check=True,
                            )
                    # normalize + transpose per 128-wide tq tile
                    otu_sb = small.tile([HS + 1, 2, 512], F32, tag="otusb",
                                        name="otusb")
                    if "norm" not in phases:
                        continue
                    nc.vector.tensor_copy(out=otu_sb, in_=otu_ps)
                    for e in range(2):
                        # softmax denominator lives in row 0; reciprocal in
                        # place, broadcast to rows 1..64 via a K=1 matmul
                        # with a ones row, then one elementwise multiply.
                        nc.vector.reciprocal(
                            out=otu_sb[HS:HS + 1, e, :],
                            in_=otu_sb[HS:HS + 1, e, :],
                        )
                        rb = ps_t.tile([HS, 512], F32, tag="tt", name="rb")
                        nc.tensor.matmul(
                            rb, ones_rows[HS:HS + 1, 0:HS],
                            otu_sb[HS:HS + 1, e, :],
                            start=True, stop=True,
                            tile_position=(64, 0),
                        )
                        otnorm = small.tile([HS, 512], BF16, tag="otnorm",
                                            name="otnorm")
                        nc.vector.tensor_mul(
                            out=otnorm,
                            in0=otu_sb[0:HS, e, :],
                            in1=rb,
                        )
                        # cross-partition placement: head parity e lives at
                        # partitions 64e..64e+64 of otn; one DMA per (p,m,e)
                        nc.sync.dma_start(
                            out=otn[64 * e:64 * e + HS, p,
                                    m * 512:(m + 1) * 512],
                            in_=otnorm,
                        )

            # ---------------- phase 3: output projection ----------------
            for tt in range(NTT if "proj" in phases else 0):
                y1 = ps_t.tile([128, 512], F32, tag="tt", name="y1")
                y2 = ps_t.tile([128, 256], F32, tag="tt", name="y2")
                for g in range(3):
                    lhs = otn[:, g, tt * 128:(tt + 1) * 128]
                    nc.tensor.matmul(
                        y1, lhs, wpt_sb[g][:, 0:512],
                        start=(g == 0), stop=(g == 2),
                    )
                    nc.tensor.matmul(
                        y2, lhs, wpt_sb[g][:, 512:768],
                        start=(g == 0), stop=(g == 2),
                    )
                ysb = ysbp.tile([128, C], F32, tag="ysb", name="ysb")
                nc.vector.tensor_copy(out=ysb[:, 0:512], in_=y1)
                nc.vector.tensor_copy(out=ysb[:, 512:768], in_=y2)
                nc.sync.dma_start(out=y[tt * 128:(tt + 1) * 128, :], in_=ysb)


_NC_CACHE = {}


def get_nc(repeat=1, phases=("v", "qk", "attn", "norm", "proj")):
    key = (repeat, tuple(phases))
    if key not in _NC_CACHE:
        nc = bacc.Bacc(
            "TRN2", target_bir_lowering=False, debug=False, num_devices=8
        )
        _NC_CACHE[key] = build_kernel(nc, repeat=repeat, phases=phases)
    return _NC_CACHE[key]


def make_in_maps(x, Wq, Wk, Wv, Wp):
    x = np.asarray(x, dtype=np.float32)
    Wq = np.asarray(Wq, dtype=np.float32)
    Wk = np.asarray(Wk, dtype=np.float32)
    Wv = np.asarray(Wv, dtype=np.float32)
    Wp = np.asarray(Wp, dtype=np.float32)
    bf = ml_dtypes.bfloat16
    in_maps = []
    for c in range(8):
        b = c // 2
        hs = HL * (c % 2)
        xT = np.ascontiguousarray(x[b].T).astype(bf)
        wqk = np.empty((HL, NCT, 128, 128), dtype=bf)
        for h in range(HL):
            stacked = np.concatenate([Wq[hs + h], Wk[hs + h]], axis=1)  # [C,128]
            for ci in range(NCT):
                wqk[h, ci] = stacked[ci * 128:(ci + 1) * 128, :].astype(bf)
        # wv: [NCT, 128, HL*HS]
        wv_full = np.transpose(Wv[hs:hs + HL], (1, 0, 2)).reshape(C, HL * HS)
        wv = np.ascontiguousarray(
            wv_full.reshape(NCT, 128, HL * HS)
        ).astype(bf)
        # wpt: Wp[:, i_slice].T -> [384, C] -> [3, 128, C]
        wpt = np.ascontiguousarray(
            Wp[:, hs * HS:(hs + HL) * HS].T.reshape(3, 128, C)
        ).astype(bf)
        in_maps.append({"xT": xT, "wqk": wqk, "wv": wv, "wpt": wpt})
    return in_maps


def run(x, Wq, Wk, Wv, Wp, bp, trace=False):
    nc = get_nc()
    in_maps = make_in_maps(x, Wq, Wk, Wv, Wp)
    res = bass_utils.run_bass_kernel_spmd(
        nc, in_maps, core_ids=list(range(8)), trace=trace
    )
    y = np.zeros((B, T, C), dtype=np.float32)
    for c in range(8):
        y[c // 2] += res.results[c]["y"]
    y += np.asarray(bp, dtype=np.float32)
    return y, res


def kernel(x, Wq, Wk, Wv, Wp, bp):
    y, _ = run(x, Wq, Wk, Wv, Wp, bp)
    return y


def make_runner(nc):
    """Build the sharded PJRT callable once (mirrors the tail of
    bass2jax.run_bass_via_pjrt) so repeated timed executions don't re-trace.
    Returns (fn, prep) where prep(in_maps) device_puts the inputs and
    fn(device_inputs) -> per-core output dicts (blocking)."""
    import jax
    from jax.experimental.shard_map import shard_map
    from jax.sharding import Mesh, PartitionSpec, NamedSharding
    from concourse import mybir as _mybir
    from concourse.bass2jax import (
        _bass_exec_p, install_neuronx_cc_hook, partition_id_tensor,
    )

    install_neuronx_cc_hook()
    n_cores = 8
    partition_name = (
        nc.partition_id_tensor.name if nc.partition_id_tensor else None
    )
    in_names, out_names, out_avals = [], [], []
    for alloc in nc.m.functions[0].allocations:
        if not isinstance(alloc, _mybir.MemoryLocationSet):
            continue
        name = alloc.memorylocations[0].name
        if alloc.kind == "ExternalInput":
            if name != partition_name:
                in_names.append(name)
        elif alloc.kind == "ExternalOutput":
            out_names.append(name)
            out_avals.append(
                jax.core.ShapedArray(
                    tuple(alloc.tensor_shape), _mybir.dt.np(alloc.dtype)
                )
            )
    n_params = len(in_names)
    n_outs = len(out_avals)
    all_in_names = in_names + out_names
    if partition_name is not None:
        all_in_names.append(partition_name)

    def _body(*args):
        operands = list(args)
        if partition_name is not None:
            operands.append(partition_id_tensor())
        outs = _bass_exec_p.bind(
            *operands,
            out_avals=tuple(out_avals),
            in_names=tuple(all_in_names),
            out_names=tuple(out_names),
            lowering_input_output_aliases=(),
            sim_require_finite=True,
            sim_require_nnan=True,
            nc=nc,
        )
        return tuple(outs)

    devices = jax.devices()[:n_cores]
    mesh = Mesh(np.array(devices), ("core",))
    sharded = jax.jit(
        shard_map(
            _body, mesh=mesh,
            in_specs=(PartitionSpec("core"),) * (n_params + n_outs),
            out_specs=(PartitionSpec("core"),) * n_outs,
            check_rep=False,
        ),
        donate_argnums=tuple(range(n_params, n_params + n_outs)),
        keep_unused=True,
    )
    shd = NamedSharding(mesh, PartitionSpec("core"))

    def prep(in_maps):
        return [
            jax.device_put(
                np.concatenate([in_maps[c][nm] for c in range(n_cores)], axis=0),
                shd,
            )
            for nm in in_names
        ]

    def zeros():
        return [
            jax.device_put(
                np.zeros((n_cores * a.shape[0], *a.shape[1:]), a.dtype), shd
            )
            for a in out_avals
        ]

    def fn(dev_inputs, dev_zeros):
        outs = sharded(*dev_inputs, *dev_zeros)
        jax.block_until_ready(outs)
        return outs

    def make_loop_fn(n_iters):
        def _body_n(*args):
            ins = args[:n_params]
            carry = tuple(args[n_params:])

            def step(i, carry):
                operands = list(ins) + list(carry)
                if partition_name is not None:
                    operands.append(partition_id_tensor())
                outs = _bass_exec_p.bind(
                    *operands,
                    out_avals=tuple(out_avals),
                    in_names=tuple(all_in_names),
                    out_names=tuple(out_names),
                    lowering_input_output_aliases=(),
                    sim_require_finite=True,
                    sim_require_nnan=True,
                    nc=nc,
                )
                return tuple(outs)

            return jax.lax.fori_loop(0, n_iters, step, carry)

        looped = jax.jit(
            shard_map(
                _body_n, mesh=mesh,
                in_specs=(PartitionSpec("core"),) * (n_params + n_outs),
                out_specs=(PartitionSpec("core"),) * n_outs,
                check_rep=False,
            ),
            donate_argnums=tuple(range(n_params, n_params + n_outs)),
            keep_unused=True,
        )

        def run_n(dev_inputs, dev_zeros):
            outs = looped(*dev_inputs, *dev_zeros)
            jax.block_until_ready(outs)
            return outs

        return run_n

    return fn, prep, zeros, out_names, make_loop_fn


# revision 16
# speedup vs baseline: 483.0950x; 1.3004x over previous
"""Multi-head causal attention (B=4, T=2048, C=768, H=12, HS=64) on 8 trn2 cores.

Sharding: 48 (batch, head) units -> 6 per core. Core c: batch c//2, heads
6*(c%2) .. 6*(c%2)+6. Each core computes a partial output projection
y_partial[T, C] = sum over its 6 heads; host sums the two partials per batch
and adds the bias.

Per-core layout choices (all chosen so softmax reductions run along the free
dim and no big transposes are needed):
  xT      [C, T]        input, pre-transposed on host, bf16
  QT/KT   [64, T] f32   per head, stored pair-stacked in 128 partitions:
                        pairQ[p][0:64]=QT_{2p}, pairQ[p][64:128]=QT_{2p+1}
  scores  ST[tk, tq]    = KT_h^T-contraction matmul(lhsT=KT, rhs=QT), so the
                        P@V matmul can consume exp(ST) directly as rhs.
  softmax               no max-subtraction (scores are O(+-8); exp is safe in
                        f32), row sums via a ones-column appended to V.
  V       vaug[tk, h, 65] f32 (col 64 = 1.0)
  P@V     OTu[65, tq] = matmul(lhsT=vaug_tile, rhs=exp(ST)) accumulated over
                        tk tiles; row 64 = softmax denominator.
  norm    per tq tile: PE-transpose OTu -> [tq, 65], reciprocal + scale,
                        PE-transpose back -> OT[d, tq], pair-stacked (bf16)
  proj    y[tq, :] = sum_g matmul(lhsT=OT[:, g, tq], rhs=WpT[g])
"""

import numpy as np
import ml_dtypes

import concourse.bacc as bacc
import concourse.bass as bass
import concourse.tile as tile
from concourse import mybir
from concourse import bass_utils

B, T, C = 4, 2048, 768
H, HS = 12, 64
HL = 6            # heads per core
NCT = C // 128    # 6 contraction tiles
NTT = T // 128    # 16 t tiles
NTC = T // 512    # 4 t chunks
NEG = -1.0e30
SCALE = 1.0 / 8.0  # 1/sqrt(HS)

F32 = mybir.dt.float32
BF16 = mybir.dt.bfloat16


def build_kernel(nc, repeat=1, phases=("v", "qk", "attn", "norm", "proj")):
    xT = nc.dram_tensor("xT", [C, T], BF16, kind="ExternalInput").ap()
    wqk = nc.dram_tensor("wqk", [HL, NCT, 128, 128], BF16, kind="ExternalInput").ap()
    wv = nc.dram_tensor("wv", [NCT, 128, HL * HS], BF16, kind="ExternalInput").ap()
    wpt = nc.dram_tensor("wpt", [3, 128, C], BF16, kind="ExternalInput").ap()
    y = nc.dram_tensor("y", [T, C], F32, kind="ExternalOutput").ap()

    with tile.TileContext(nc) as tc:
        with (
            tc.tile_pool(name="consts", bufs=1) as consts,
            tc.tile_pool(name="xw", bufs=1) as xw,
            tc.tile_pool(name="qk_stage", bufs=2) as qk_stage,
            tc.tile_pool(name="pt", bufs=16) as ptp,
            tc.tile_pool(name="small", bufs=3) as small,
            tc.tile_pool(name="ysb", bufs=2) as ysbp,
            # PSUM: st2 (2 slots x 2 banks) + otu2 (1 slot x 2 banks) +
            # tt (2 slots x 1 bank, shared by psv/psqk/t1/t2/y1/y2) = 8 banks
            tc.tile_pool(name="ps_st", bufs=2, space="PSUM") as ps_st,
            tc.tile_pool(name="ps_otu", bufs=1, space="PSUM") as ps_otu,
            tc.tile_pool(name="ps_t", bufs=2, space="PSUM") as ps_t,
        ):
            # ---------------- weights + x ----------------
            xt = []
            for ci in range(NCT):
                t_ = xw.tile([128, T], BF16, tag=f"xt{ci}", name=f"xt{ci}")
                nc.sync.dma_start(out=t_, in_=xT[ci * 128:(ci + 1) * 128, :])
                xt.append(t_)
            wqk_sb = []
            for h in range(HL):
                row = []
                for ci in range(NCT):
                    t_ = xw.tile([128, 128], BF16, tag=f"wqk{h}_{ci}",
                                 name=f"wqk{h}_{ci}")
                    nc.sync.dma_start(out=t_, in_=wqk[h, ci])
                    row.append(t_)
                wqk_sb.append(row)
            wv_sb = []
            for ci in range(NCT):
                t_ = xw.tile([128, HL * HS], BF16, tag=f"wv{ci}", name=f"wv{ci}")
                nc.sync.dma_start(out=t_, in_=wv[ci])
                wv_sb.append(t_)
            wpt_sb = []
            for g in range(3):
                t_ = consts.tile([128, C], BF16, tag=f"wpt{g}", name=f"wpt{g}")
                nc.sync.dma_start(out=t_, in_=wpt[g])
                wpt_sb.append(t_)

            # persistent tensors (allocated once; loop iterations rewrite)
            vaug = consts.tile([128, NTT, HL, HS + 1], BF16)
            nc.gpsimd.memset(vaug[:, :, :, HS:HS + 1], 1.0)
            pairQ = [consts.tile([128, T], BF16, tag=f"pq{p}", name=f"pq{p}")
                     for p in range(3)]
            pairK = [consts.tile([128, T], BF16, tag=f"pk{p}", name=f"pk{p}")
                     for p in range(3)]
            otn = consts.tile([128, 3, T], BF16)
            if "v" not in phases:
                nc.gpsimd.memset(vaug[:, :, :, 0:HS], 0.0)
            if "qk" not in phases:
                for p in range(3):
                    nc.gpsimd.memset(pairQ[p], 0.0)
                    nc.gpsimd.memset(pairK[p], 0.0)
            if "norm" not in phases:
                nc.gpsimd.memset(otn, 0.0)

            import contextlib
            rep_ctx = (
                tc.For_i(0, repeat, 1,
                         hint_engines=(mybir.EngineType.PE,
                                       mybir.EngineType.DVE,
                                       mybir.EngineType.Activation,
                                       mybir.EngineType.SP,
                                       mybir.EngineType.Pool))
                if repeat > 1 else contextlib.nullcontext()
            )
            with rep_ctx:
                build_phases(nc, tc, consts, xw, qk_stage, ptp, small, ysbp,
                             ps_st, ps_otu, ps_t,
                             xt, wqk_sb, wv_sb, wpt_sb,
                             vaug, pairQ, pairK, otn, y, phases)

    nc.compile()
    return nc


def build_phases(nc, tc, consts, xw, qk_stage, ptp, small, ysbp,
                 ps_st, ps_otu, ps_t,
                 xt, wqk_sb, wv_sb, wpt_sb,
                 vaug, pairQ, pairK, otn, y,
                 phases=("v", "qk", "attn", "norm", "proj")):
    if True:
        if True:
            ones_rows = consts.tile([128, HS + 1], F32)
            nc.gpsimd.memset(ones_rows, 1.0)

            # ---------------- phase 1a: V projection ----------------
            # vaug[:, tt, h, 0:64] = V tile, [..., 64] = 1.0
            for tt in range(NTT if "v" in phases else 0):
                ps = ps_t.tile([128, HL * HS], F32, tag="tt", name="psv")
                for ci in range(NCT):
                    nc.tensor.matmul(
                        ps, xt[ci][:, tt * 128:(tt + 1) * 128], wv_sb[ci],
                        start=(ci == 0), stop=(ci == NCT - 1),
                    )
                nc.vector.tensor_copy(
                    out=vaug[:, tt, :, 0:HS],
                    in_=ps.rearrange("p (h d) -> p h d", h=HL),
                )

            # ---------------- phase 1b: Q/K projections ----------------
            # pair p holds heads (2p, 2p+1); head parity e lives at
            # partitions 64e..64e+64 of pairQ/pairK.
            for h in range(HL if "qk" in phases else 0):
                p, e = divmod(h, 2)
                stg = qk_stage.tile([128, T], BF16, tag="qkstg", name="qkstg")
                for m in range(NTC):
                    sl = slice(m * 512, (m + 1) * 512)
                    ps = ps_t.tile([128, 512], F32, tag="tt", name="psqk")
                    for ci in range(NCT):
                        nc.tensor.matmul(
                            ps, wqk_sb[h][ci], xt[ci][:, sl],
                            start=(ci == 0), stop=(ci == NCT - 1),
                        )
                    nc.vector.tensor_copy(out=stg[:, sl], in_=ps)
                # Q half -> pairQ[p][64e:64e+64], K half -> pairK same rows
                nc.sync.dma_start(
                    out=pairQ[p][64 * e:64 * e + 64, :], in_=stg[0:64, :]
                )
                nc.sync.dma_start(
                    out=pairK[p][64 * e:64 * e + 64, :], in_=stg[64:128, :]
                )

            # ---------------- phase 2+3: attention & projection ----------
            # m-chunk outer so the output projection of chunk m overlaps the
            # attention of chunk m+1.
            do_attn = bool({"attn", "norm", "sc", "exp", "otu"} & set(phases))
            for m in range(NTC if (do_attn or "proj" in phases) else 0):
                jmax = 4 * m + 3
                do_sc = "attn" in phases or "sc" in phases
                do_exp = "attn" in phases or "exp" in phases
                do_otu = "attn" in phases or "otu" in phases
                for p in range(3 if do_attn else 0):
                    otu_ps = ps_otu.tile([HS + 1, 2, 512], F32, tag="otu",
                                         name="otu")
                    if "norm" in phases and not (
                        "attn" in phases or "otu" in phases
                    ):
                        nc.vector.memset(otu_ps, 1.0)
                    pts = []
                    for j in range((jmax + 1) if do_sc else 0):
                        s0 = max(0, j - 4 * m)
                        st = ps_st.tile([128, 2, 512], F32, tag="st", name="st")
                        for e in range(2):
                            nc.tensor.matmul(
                                st[:, e, 128 * s0:512],
                                pairK[p][64 * e:64 * e + 64,
                                         j * 128:(j + 1) * 128],
                                pairQ[p][64 * e:64 * e + 64,
                                         m * 512 + 128 * s0:(m + 1) * 512],
                                start=True, stop=True,
                                tile_position=(64 * e, 0),
                            )
                        if not do_exp:
                            continue
                        pt = ptp.tile([128, 2, 512], BF16, tag="pt", name="pt")
                        pts.append(pt)
                        # one fused exp over both heads (2-bank strided AP)
                        nc.scalar.activation(
                            out=pt[:, :, 128 * s0:512],
                            in_=st[:, :, 128 * s0:512],
                            func=mybir.ActivationFunctionType.Exp,
                            scale=SCALE,
                        )
                        if j >= 4 * m:
                            # zero the below-diagonal triangle of the diagonal
                            # subtile for both heads (keep where tq >= tk)
                            nc.gpsimd.affine_select(
                                out=pt[:, :, 128 * s0:128 * s0 + 128],
                                in_=pt[:, :, 128 * s0:128 * s0 + 128],
                                compare_op=mybir.AluOpType.is_ge,
                                fill=0.0, base=0,
                                pattern=[[0, 2], [1, 128]],
                                channel_multiplier=-1,
                            )
                    # separate P@V loop: PE never stalls on ACT here
                    for j in range((jmax + 1) if do_otu else 0):
                        s0 = max(0, j - 4 * m)
                        for e in range(2):
                            nc.tensor.matmul(
                                otu_ps[:, e, 128 * s0:512],
                                vaug[:, j, 2 * p + e, :],
                                pts[j][:, e, 128 * s0:512],
                                start=(j == 0), stop=(j == jmax),
                                skip_group_check=True,
                            )
                    # normalize per head
                    otu_sb = small.tile([HS + 1, 2, 512], F32, tag="otusb",
                                        name="otusb")
                    if "norm" not in phases:
                        continue
                    nc.vector.tensor_copy(out=otu_sb, in_=otu_ps)
                    for e in range(2):
                        # softmax denominator lives in row HS; reciprocal in
                        # place, broadcast to rows 0..63 via a K=1 matmul
                        # with a ones row, then one elementwise multiply.
                        nc.vector.reciprocal(
                            out=otu_sb[HS:HS + 1, e, :],
                            in_=otu_sb[HS:HS + 1, e, :],
                        )
                        rb = ps_t.tile([HS, 512], F32, tag="tt", name="rb")
                        nc.tensor.matmul(
                            rb, ones_rows[HS:HS + 1, 0:HS],
                            otu_sb[HS:HS + 1, e, :],
                            start=True, stop=True,
                            tile_position=(64, 0),
                        )
                        otnorm = small.tile([HS, 512], BF16, tag="otnorm",
                                            name="otnorm")
                        nc.vector.tensor_mul(
                            out=otnorm,
                            in0=otu_sb[0:HS, e, :],
                            in1=rb,
                        )
                        # cross-partition placement: head parity e lives at
                        # partitions 64e..64e+64 of otn; one DMA per (p,m,e)
                        nc.sync.dma_start(
                            out=otn[64 * e:64 * e + HS, p,
                                    m * 512:(m + 1) * 512],
                            in_=otnorm,
                        )

                # ---- output projection for this m-chunk's 4 tq tiles ----
                for tt in range(4 * m, 4 * m + 4):
                    if "proj" not in phases:
                        continue
                    y1 = ps_t.tile([128, 512], F32, tag="tt", name="y1")
                    y2 = ps_t.tile([128, 256], F32, tag="tt", name="y2")
                    for g in range(3):
                        lhs = otn[:, g, tt * 128:(tt + 1) * 128]
                        nc.tensor.matmul(
                            y1, lhs, wpt_sb[g][:, 0:512],
                            start=(g == 0), stop=(g == 2),
                        )
                        nc.tensor.matmul(
                            y2, lhs, wpt_sb[g][:, 512:768],
                            start=(g == 0), stop=(g == 2),
                        )
                    ysb = ysbp.tile([128, C], F32, tag="ysb", name="ysb")
                    nc.vector.tensor_copy(out=ysb[:, 0:512], in_=y1)
                    nc.vector.tensor_copy(out=ysb[:, 512:768], in_=y2)
                    nc.sync.dma_start(out=y[tt * 128:(tt + 1) * 128, :], in_=ysb)


_NC_CACHE = {}


def get_nc(repeat=1, phases=("v", "qk", "attn", "norm", "proj")):
    key = (repeat, tuple(phases))
    if key not in _NC_CACHE:
        nc = bacc.Bacc(
            "TRN2", target_bir_lowering=False, debug=False, num_devices=8
        )
        _NC_CACHE[key] = build_kernel(nc, repeat=repeat, phases=phases)
    return _NC_CACHE[key]


def make_in_maps(x, Wq, Wk, Wv, Wp):
    x = np.asarray(x, dtype=np.float32)
    Wq = np.asarray(Wq, dtype=np.float32)
    Wk = np.asarray(Wk, dtype=np.float32)
    Wv = np.asarray(Wv, dtype=np.float32)
    Wp = np.asarray(Wp, dtype=np.float32)
    bf = ml_dtypes.bfloat16
    in_maps = []
    for c in range(8):
        b = c // 2
        hs = HL * (c % 2)
        xT = np.ascontiguousarray(x[b].T).astype(bf)
        wqk = np.empty((HL, NCT, 128, 128), dtype=bf)
        for h in range(HL):
            stacked = np.concatenate([Wq[hs + h], Wk[hs + h]], axis=1)  # [C,128]
            for ci in range(NCT):
                wqk[h, ci] = stacked[ci * 128:(ci + 1) * 128, :].astype(bf)
        # wv: [NCT, 128, HL*HS]
        wv_full = np.transpose(Wv[hs:hs + HL], (1, 0, 2)).reshape(C, HL * HS)
        wv = np.ascontiguousarray(
            wv_full.reshape(NCT, 128, HL * HS)
        ).astype(bf)
        # wpt: Wp[:, i_slice].T -> [384, C] -> [3, 128, C]
        wpt = np.ascontiguousarray(
            Wp[:, hs * HS:(hs + HL) * HS].T.reshape(3, 128, C)
        ).astype(bf)
        in_maps.append({"xT": xT, "wqk": wqk, "wv": wv, "wpt": wpt})
    return in_maps


def run(x, Wq, Wk, Wv, Wp, bp, trace=False):
    nc = get_nc()
    in_maps = make_in_maps(x, Wq, Wk, Wv, Wp)
    res = bass_utils.run_bass_kernel_spmd(
        nc, in_maps, core_ids=list(range(8)), trace=trace
    )
    y = np.zeros((B, T, C), dtype=np.float32)
    for c in range(8):
        y[c // 2] += res.results[c]["y"]
    y += np.asarray(bp, dtype=np.float32)
    return y, res


def kernel(x, Wq, Wk, Wv, Wp, bp):
    y, _ = run(x, Wq, Wk, Wv, Wp, bp)
    return y


def make_runner(nc):
    """Build the sharded PJRT callable once (mirrors the tail of
    bass2jax.run_bass_via_pjrt) so repeated timed executions don't re-trace.
    Returns (fn, prep) where prep(in_maps) device_puts the inputs and
    fn(device_inputs) -> per-core output dicts (blocking)."""
    import jax
    from jax.experimental.shard_map import shard_map
    from jax.sharding import Mesh, PartitionSpec, NamedSharding
    from concourse import mybir as _mybir
    from concourse.bass2jax import (
        _bass_exec_p, install_neuronx_cc_hook, partition_id_tensor,
    )

    install_neuronx_cc_hook()
    n_cores = 8
    partition_name = (
        nc.partition_id_tensor.name if nc.partition_id_tensor else None
    )
    in_names, out_names, out_avals = [], [], []
    for alloc in nc.m.functions[0].allocations:
        if not isinstance(alloc, _mybir.MemoryLocationSet):
            continue
        name = alloc.memorylocations[0].name
        if alloc.kind == "ExternalInput":
            if name != partition_name:
                in_names.append(name)
        elif alloc.kind == "ExternalOutput":
            out_names.append(name)
            out_avals.append(
                jax.core.ShapedArray(
                    tuple(alloc.tensor_shape), _mybir.dt.np(alloc.dtype)
                )
            )
    n_params = len(in_names)
    n_outs = len(out_avals)
    all_in_names = in_names + out_names
    if partition_name is not None:
        all_in_names.append(partition_name)

    def _body(*args):
        operands = list(args)
        if partition_name is not None:
            operands.append(partition_id_tensor())
        outs = _bass_exec_p.bind(
            *operands,
            out_avals=tuple(out_avals),
            in_names=tuple(all_in_names),
            out_names=tuple(out_names),
            lowering_input_output_aliases=(),
            sim_require_finite=True,
            sim_require_nnan=True,
            nc=nc,
        )
        return tuple(outs)

    devices = jax.devices()[:n_cores]
    mesh = Mesh(np.array(devices), ("core",))
    sharded = jax.jit(
        shard_map(
            _body, mesh=mesh,
            in_specs=(PartitionSpec("core"),) * (n_params + n_outs),
            out_specs=(PartitionSpec("core"),) * n_outs,
            check_rep=False,
        ),
        donate_argnums=tuple(range(n_params, n_params + n_outs)),
        keep_unused=True,
    )
    shd = NamedSharding(mesh, PartitionSpec("core"))

    def prep(in_maps):
        return [
            jax.device_put(
                np.concatenate([in_maps[c][nm] for c in range(n_cores)], axis=0),
                shd,
            )
            for nm in in_names
        ]

    def zeros():
        return [
            jax.device_put(
                np.zeros((n_cores * a.shape[0], *a.shape[1:]), a.dtype), shd
            )
            for a in out_avals
        ]

    def fn(dev_inputs, dev_zeros):
        outs = sharded(*dev_inputs, *dev_zeros)
        jax.block_until_ready(outs)
        return outs

    def make_loop_fn(n_iters):
        def _body_n(*args):
            ins = args[:n_params]
            carry = tuple(args[n_params:])

            def step(i, carry):
                operands = list(ins) + list(carry)
                if partition_name is not None:
                    operands.append(partition_id_tensor())
                outs = _bass_exec_p.bind(
                    *operands,
                    out_avals=tuple(out_avals),
                    in_names=tuple(all_in_names),
                    out_names=tuple(out_names),
                    lowering_input_output_aliases=(),
                    sim_require_finite=True,
                    sim_require_nnan=True,
                    nc=nc,
                )
                return tuple(outs)

            return jax.lax.fori_loop(0, n_iters, step, carry)

        looped = jax.jit(
            shard_map(
                _body_n, mesh=mesh,
                in_specs=(PartitionSpec("core"),) * (n_params + n_outs),
                out_specs=(PartitionSpec("core"),) * n_outs,
                check_rep=False,
            ),
            donate_argnums=tuple(range(n_params, n_params + n_outs)),
            keep_unused=True,
        )

        def run_n(dev_inputs, dev_zeros):
            outs = looped(*dev_inputs, *dev_zeros)
            jax.block_until_ready(outs)
            return outs

        return run_n

    return fn, prep, zeros, out_names, make_loop_fn
